# revision 36
# baseline (speedup 1.0000x reference)
"""2-layer GCN on 8 Trainium2 NeuronCores — aggregate-then-project.

Nodes are range-sharded across 8 cores (dst parallel). Both GCN layers are
computed as: gather source rows from a bf16 feature table (dma_gather with
2 src buckets, biased signed-int16 indices), segment-sum via one-hot bf16
matmuls into PSUM, then per-dst-tile projection:

  layer1 table = dinv*x (host-precomputed, full -> no collective needed)
      aggx[d]  = sum_{s->d} table1[s]          (self-loops in edge list)
      t2[d]    = dinv[d] * relu(dinv[d]*(aggx[d] @ W1) + b1)
  AllGather t2 -> table2
      out[d]   = dinv[d]*(agg2[d] @ W2) + b2

Gathers are batched per group of 4 dst tiles, 4 sub-gathers per group
round-robined over the 4 SWDGE queues (descriptor generation runs on the
Q7 core pair owning each queue, so queues overlap ~2-3.5x).
"""
import os
import sys

sys.path.insert(0, "/opt/trn_rl_repo")

import numpy as np
import ml_dtypes

import concourse.bass as bass
import concourse.bacc as bacc
import concourse.tile as tile
import concourse.mybir as mybir
from concourse import bass_utils
from concourse.library_config import mlp

N_CORES = 8
N_NODES = 100000
D_IN, D_H, D_OUT = 128, 64, 64
NSHARD = N_NODES // N_CORES          # 12500
TILE = 128
NT = (NSHARD + TILE - 1) // TILE     # 98
PADN = NT * TILE                     # 12544
PADN_ALL = N_CORES * PADN            # 100352
N_BUCKET = 2
B0_ROWS = 50176
BIAS = (32768, B0_ROWS + 32768)
GROUP = 4
GROUPS = [(t, min(t + GROUP, NT)) for t in range(0, NT, GROUP)]
N_GROUP = len(GROUPS)                # 25 (24x4 tiles + 1x2)

LAST_RESULT = None


def _host_prep(x, edge_index):
    src = np.asarray(edge_index[0], dtype=np.int64)
    dst = np.asarray(edge_index[1], dtype=np.int64)
    n = N_NODES

    deg = np.bincount(dst, minlength=n).astype(np.float64) + 1.0
    dinv = (1.0 / np.sqrt(deg)).astype(np.float32)

    # self-loops are applied densely on-device (one identity matmul per
    # tile) instead of occupying gather slots
    s_all = src
    d_all = dst

    core = d_all // NSHARD
    drem = d_all % NSHARD
    t_id = drem // TILE
    dloc = drem % TILE
    gsrc = (s_all // NSHARD) * PADN + (s_all % NSHARD)
    bkt = (gsrc >= B0_ROWS).astype(np.int64)

    key = (core * NT + t_id) * N_BUCKET + bkt
    order = np.argsort(key, kind="stable")
    key_s = key[order]
    gsrc_s = gsrc[order]
    dloc_s = dloc[order]

    ngroups = N_CORES * NT * N_BUCKET
    counts = np.bincount(key_s, minlength=ngroups).reshape(N_CORES, NT, N_BUCKET)
    nb = -(-counts.max(axis=0) // 128)                  # [NT, N_BUCKET] ceil
    nb = np.maximum(nb, 1)
    # each sub-gather ends at (tmid-1, b) or (t1-1, b); those (t,b) must end
    # with >=1 pad slot on every core (the gather ucode trims trailing
    # negative idxs, which would otherwise drop real edges).
    for (t0, t1) in GROUPS:
        tmid = (t0 + t1) // 2
        for tf in (tmid - 1, t1 - 1):
            for b in range(N_BUCKET):
                if (counts[:, tf, b] == nb[tf, b] * 128).any():
                    nb[tf, b] += 1

    # slot layout: per group of GROUP tiles: all b0 chunks (tile-major), then
    # all b1 chunks -> one contiguous gather dst region per (group, bucket).
    chunk_col = np.zeros((NT, N_BUCKET), np.int64)
    grp_nc = np.zeros(N_GROUP, np.int64)
    grp_base = np.zeros(N_GROUP, np.int64)
    grp_b_off = np.zeros((N_GROUP, N_BUCKET + 1), np.int64)
    pos = 0
    for g, (t0, t1) in enumerate(GROUPS):
        grp_base[g] = pos
        for b in range(N_BUCKET):
            grp_b_off[g, b] = pos - grp_base[g]
            for t in range(t0, t1):
                chunk_col[t, b] = pos
                pos += nb[t, b]
        grp_nc[g] = pos - grp_base[g]
        grp_b_off[g, N_BUCKET] = grp_nc[g]
    CHC = pos
    IDXC16 = CHC * 8

    grp_start = np.zeros(ngroups + 1, np.int64)
    np.cumsum(counts.reshape(-1), out=grp_start[1:])
    rank = np.arange(key_s.shape[0], dtype=np.int64) - grp_start[key_s]

    core_s = key_s // (NT * N_BUCKET)
    tb = key_s % (NT * N_BUCKET)
    t_s = tb // N_BUCKET
    b_s = tb % N_BUCKET

    slot = chunk_col[t_s, b_s] * 128 + rank
    ccol = slot // 128
    cpart = slot % 128

    idx_val = gsrc_s - np.where(b_s == 0, BIAS[0], BIAS[1])
    assert idx_val.min() >= -32768 and idx_val.max() <= 32767

    idx16_16 = np.zeros((N_CORES, 16, IDXC16), np.int16)
    idx16_16[core_s, slot % 16, slot // 16] = idx_val.astype(np.int16)
    idx16 = np.tile(idx16_16, (1, 8, 1))                # [cores, 128, IDXC16]

    dstloc = np.full((N_CORES, 128, CHC), 999.0, np.float32)
    dstloc[core_s, cpart, ccol] = dloc_s.astype(np.float32)

    dinv_cols = np.zeros((N_CORES, 128, NT), np.float32)
    node_grid = (
        np.arange(N_CORES)[:, None, None] * NSHARD
        + np.arange(NT)[None, None, :] * TILE
        + np.arange(128)[None, :, None]
    )
    local = np.arange(NT)[None, None, :] * TILE + np.arange(128)[None, :, None]
    valid = np.broadcast_to(local < NSHARD, node_grid.shape)
    dinv_cols[:] = np.where(valid, dinv[np.where(valid, node_grid, 0)], 0.0)

    xt = np.zeros((PADN_ALL, D_IN), np.float32)
    xs = (np.asarray(x, np.float32) * dinv[:, None]).reshape(N_CORES, NSHARD, D_IN)
    xt.reshape(N_CORES, PADN, D_IN)[:, :NSHARD] = xs
    xt = xt.astype(ml_dtypes.bfloat16)

    meta = dict(nb=nb, chunk_col=chunk_col, grp_nc=grp_nc, grp_base=grp_base,
                grp_b_off=grp_b_off, CHC=CHC, IDXC16=IDXC16)
    xts = np.ascontiguousarray(xt.reshape(N_CORES, PADN, D_IN))
    return xt, xts, idx16, dstloc, dinv_cols, meta


def _build_program(meta, b1_zero, b2_zero):
    nb = meta["nb"]
    chunk_col = meta["chunk_col"]
    grp_nc = meta["grp_nc"]
    grp_base = meta["grp_base"]
    grp_b_off = meta["grp_b_off"]
    CHC, IDXC16 = meta["CHC"], meta["IDXC16"]

    f32 = mybir.dt.float32
    bf16 = mybir.dt.bfloat16
    i16 = mybir.dt.int16
    nc = bacc.Bacc("TRN2", target_bir_lowering=False, debug=False,
                   num_devices=N_CORES, num_swdge_queues=4)

    xt_in = nc.dram_tensor("xt", [PADN_ALL, D_IN], bf16, kind="ExternalInput").ap()
    xs_in = nc.dram_tensor("xts", [PADN, D_IN], bf16, kind="ExternalInput").ap()
    idb_in = nc.dram_tensor("identb", [128, 128], bf16, kind="ExternalInput").ap()
    w1_in = nc.dram_tensor("W1", [D_IN, D_H], f32, kind="ExternalInput").ap()
    w2_in = nc.dram_tensor("W2", [D_H, D_OUT], f32, kind="ExternalInput").ap()
    b1_in = nc.dram_tensor("b1r", [128, D_H], f32, kind="ExternalInput").ap()
    b2_in = nc.dram_tensor("b2r", [128, D_OUT], f32, kind="ExternalInput").ap()
    io_in = nc.dram_tensor("iota", [128, 128], f32, kind="ExternalInput").ap()
    dv_in = nc.dram_tensor("dinv_cols", [128, NT], f32, kind="ExternalInput").ap()
    ix_in = nc.dram_tensor("idx16", [128, IDXC16], i16, kind="ExternalInput").ap()
    dl_in = nc.dram_tensor("dstloc", [128, CHC], f32, kind="ExternalInput").ap()
    out_t = nc.dram_tensor("out", [PADN, D_OUT], f32, kind="ExternalOutput").ap()

    rg = [list(range(N_CORES))]

    with tile.TileContext(nc) as tc:
        with tc.tile_pool(name="const", bufs=1) as constp, \
             tc.tile_pool(name="dram", bufs=1, space="DRAM") as dram, \
             tc.tile_pool(name="agg", bufs=2, space="PSUM") as aggp, \
             tc.tile_pool(name="tp", bufs=2, space="PSUM") as tpp, \
             tc.tile_pool(name="proj", bufs=2, space="PSUM") as projp, \
             tc.tile_pool(name="gat", bufs=5) as gatp, \
             tc.tile_pool(name="sel", bufs=2) as selp, \
             tc.tile_pool(name="sb", bufs=3) as sb:

            nc.gpsimd.load_library(mlp)

            w1 = constp.tile([D_IN, D_H], f32)
            nc.sync.dma_start(w1[:], w1_in[:])
            w2 = constp.tile([D_H, D_OUT], f32)
            nc.sync.dma_start(w2[:], w2_in[:])
            b1r = constp.tile([128, D_H], f32)
            nc.sync.dma_start(b1r[:], b1_in[:])
            b2r = constp.tile([128, D_OUT], f32)
            nc.sync.dma_start(b2r[:], b2_in[:])
            iota = constp.tile([128, 128], f32)
            nc.sync.dma_start(iota[:], io_in[:])
            identb = constp.tile([128, 128], bf16)
            nc.sync.dma_start(identb[:], idb_in[:])
            dvc = constp.tile([128, NT], f32)
            nc.sync.dma_start(dvc[:], dv_in[:])
            # idx + dstloc resident for both layers (same edges both layers)
            ixall = constp.tile([128, IDXC16], i16)
            nc.sync.dma_start(ixall[:], ix_in[:])
            dlall = constp.tile([128, CHC], f32)
            nc.sync.dma_start(dlall[:], dl_in[:])

            t2_shard = dram.tile([PADN, D_IN], bf16)
            t2_full = dram.tile([PADN_ALL, D_IN], bf16)

            def edge_pass(table_ap, width, layer):
                """table_ap: [PADN_ALL, 128] bf16 AP. width: 128 or 64."""
                tb0 = table_ap[BIAS[0]:B0_ROWS, :]
                tb1 = table_ap[BIAS[1]:PADN_ALL, :]
                def issue_gathers(g, t0g, t1g):
                    base = int(grp_base[g])
                    ncg = int(grp_nc[g])
                    G = gatp.tile([128, ncg, D_IN], bf16, tag="G")
                    # 4 sub-gathers per group (one per SWDGE queue): each
                    # bucket's chunk range split at a tile boundary so every
                    # sub-gather still ends in pad slots (trailing-trim safe).
                    qn = 2 * g
                    for b in range(N_BUCKET):
                        c0 = int(grp_b_off[g, b])
                        c1 = int(grp_b_off[g, b + 1])
                        ncb = c1 - c0
                        if ncb == 0:
                            continue
                        nidx = ncb * 128
                        nc.gpsimd.dma_gather(
                            G[:, c0:c1, :],
                            tb0 if b == 0 else tb1,
                            ixall[:, (base + c0) * 8:(base + c1) * 8],
                            nidx, nidx, D_IN,
                            single_packet=False,
                            queue_num=qn % 4,
                        )
                        qn += 1
                    return G

                def consume_group(g, t0g, t1g, G):
                    base = int(grp_base[g])
                    for t in range(t0g, t1g):
                        nb0 = int(nb[t, 0])
                        nb1 = int(nb[t, 1])
                        nct = nb0 + nb1
                        l0 = int(chunk_col[t, 0]) - base
                        l1 = int(chunk_col[t, 1]) - base
                        dxt = sb.tile([128, width], bf16, tag="dx")
                        if layer == 1:
                            nc.sync.dma_start(
                                dxt[:], xs_in[t * 128:(t + 1) * 128, :])
                        else:
                            nc.sync.dma_start(
                                dxt[:],
                                t2_shard[t * 128:(t + 1) * 128, 0:D_H])
                        S0 = selp.tile([128, nb0, 128], bf16, tag="S0")
                        nc.vector.tensor_tensor(
                            out=S0[:],
                            in0=dlall[:, base + l0:base + l0 + nb0]
                                .to_broadcast([128, nb0, 128]),
                            in1=iota[:].unsqueeze(1).to_broadcast([128, nb0, 128]),
                            op=mybir.AluOpType.is_equal,
                        )
                        S1 = selp.tile([128, nb1, 128], bf16, tag="S1")
                        nc.vector.tensor_tensor(
                            out=S1[:],
                            in0=dlall[:, base + l1:base + l1 + nb1]
                                .to_broadcast([128, nb1, 128]),
                            in1=iota[:].unsqueeze(1).to_broadcast([128, nb1, 128]),
                            op=mybir.AluOpType.is_equal,
                        )
                        # aggT[f, d] = sum_e G[e, f] * S[e, d]  (G stationary,
                        # S moving) -> agg arrives pre-transposed for the
                        # feature-contraction projection matmul: no transpose.
                        aggT = aggp.tile([width, 128], f32, tag="agg")
                        # self-loop: aggT += dxt.T @ I (dense local rows)
                        nc.tensor.matmul(aggT[:], lhsT=dxt[:], rhs=identb[:],
                                         start=True, stop=False)
                        for i in range(nct):
                            S = S0[:, i, :] if i < nb0 else S1[:, i - nb0, :]
                            gc = (l0 + i) if i < nb0 else (l1 + i - nb0)
                            nc.tensor.matmul(
                                aggT[:], lhsT=G[:, gc, 0:width],
                                rhs=S,
                                start=False, stop=(i == nct - 1),
                            )
                        aggT_sb = sb.tile([width, 128], f32, tag="e1")
                        nc.scalar.copy(aggT_sb[:], aggT[:])
                        if layer == 1:
                            # t2 = dinv*relu(dinv*(agg @ W1) + b1)
                            proj = projp.tile([128, D_H], f32, tag="proj")
                            nc.tensor.matmul(proj[:], lhsT=aggT_sb[:], rhs=w1[:],
                                             start=True, stop=True)
                            if b1_zero:
                                hr = sb.tile([128, D_H], f32, tag="e5")
                                nc.scalar.activation(
                                    hr[:], proj[:],
                                    mybir.ActivationFunctionType.Relu,
                                    scale=dvc[:, t:t + 1])
                            else:
                                hv = sb.tile([128, D_H], f32, tag="e3")
                                nc.scalar.activation(
                                    hv[:], proj[:],
                                    mybir.ActivationFunctionType.Copy,
                                    scale=dvc[:, t:t + 1])
                                hb = sb.tile([128, D_H], f32, tag="e4")
                                nc.vector.tensor_add(hb[:], hv[:], b1r[:])
                                hr = sb.tile([128, D_H], f32, tag="e5")
                                nc.scalar.activation(
                                    hr[:], hb[:],
                                    mybir.ActivationFunctionType.Relu)
                            t2t = sb.tile([128, D_H], bf16, tag="e6")
                            nc.scalar.activation(
                                t2t[:], hr[:],
                                mybir.ActivationFunctionType.Copy,
                                scale=dvc[:, t:t + 1])
                            nc.sync.dma_start(
                                t2_shard[t * 128:(t + 1) * 128, 0:D_H], t2t[:])
                        else:
                            # out = dinv*(agg @ W2) + b2
                            o_ps = projp.tile([128, D_OUT], f32, tag="proj")
                            nc.tensor.matmul(o_ps[:], lhsT=aggT_sb[:], rhs=w2[:],
                                             start=True, stop=True)
                            ov = sb.tile([128, D_OUT], f32, tag="e3")
                            nc.scalar.activation(
                                ov[:], o_ps[:],
                                mybir.ActivationFunctionType.Copy,
                                scale=dvc[:, t:t + 1])
                            if b2_zero:
                                nc.sync.dma_start(
                                    out_t[t * 128:(t + 1) * 128, :], ov[:])
                            else:
                                ob = sb.tile([128, D_OUT], f32, tag="e4")
                                nc.vector.tensor_add(ob[:], ov[:], b2r[:])
                                nc.sync.dma_start(
                                    out_t[t * 128:(t + 1) * 128, :], ob[:])

                for gp in range(N_GROUP):
                    ga = issue_gathers(gp, *GROUPS[gp])
                    consume_group(gp, *GROUPS[gp], ga)

            edge_pass(xt_in, 128, layer=1)

            nc.gpsimd.collective_compute(
                "AllGather", mybir.AluOpType.bypass,
                ins=[t2_shard.opt()], outs=[t2_full.opt()],
                replica_groups=rg,
            )

            edge_pass(t2_full[:], D_H, layer=2)

    nc.compile()
    return nc


def kernel(x, edge_index, W1, b1, W2, b2):
    global LAST_RESULT
    x = np.asarray(x, np.float32)
    W1 = np.asarray(W1, np.float32)
    W2 = np.asarray(W2, np.float32)
    b1 = np.asarray(b1, np.float32)
    b2 = np.asarray(b2, np.float32)

    xt, xts, idx16, dstloc, dinv_cols, meta = _host_prep(x, edge_index)
    nc = _build_program(meta, bool(np.all(b1 == 0.0)),
                        bool(np.all(b2 == 0.0)))

    iota = np.tile(np.arange(128, dtype=np.float32), (128, 1))
    identb = np.eye(128, dtype=np.float32).astype(ml_dtypes.bfloat16)
    b1r = np.tile(b1[None, :], (128, 1)).astype(np.float32)
    b2r = np.tile(b2[None, :], (128, 1)).astype(np.float32)

    in_maps = []
    for k in range(N_CORES):
        in_maps.append({
            "xt": xt, "xts": xts[k], "identb": identb,
            "W1": W1, "W2": W2, "b1r": b1r, "b2r": b2r,
            "iota": iota,
            "dinv_cols": dinv_cols[k],
            "idx16": idx16[k],
            "dstloc": dstloc[k],
        })

    trace = bool(os.environ.get("BASS_TRACE"))
    res = bass_utils.run_bass_kernel_spmd(
        nc, in_maps, core_ids=list(range(N_CORES)), trace=trace)
    LAST_RESULT = res

    out = np.empty((N_NODES, D_OUT), np.float32)
    for k in range(N_CORES):
        out[k * NSHARD:(k + 1) * NSHARD] = res.results[k]["out"][:NSHARD]
    return out


# revision 37
# speedup vs baseline: 1.1481x; 1.1481x over previous
"""2-layer GCN on 8 Trainium2 NeuronCores — aggregate-then-project.

Nodes are range-sharded across 8 cores (dst parallel). Both GCN layers are
computed as: gather source rows from a bf16 feature table (dma_gather with
2 src buckets, biased signed-int16 indices), segment-sum via one-hot bf16
matmuls into PSUM, then per-dst-tile projection:

  layer1 table = dinv*x (host-precomputed, full -> no collective needed)
      aggx[d]  = sum_{s->d} table1[s]          (self-loops in edge list)
      t2[d]    = dinv[d] * relu(dinv[d]*(aggx[d] @ W1) + b1)
  AllGather t2 -> table2
      out[d]   = dinv[d]*(agg2[d] @ W2) + b2

Gathers are batched per group of 4 dst tiles, 4 sub-gathers per group
round-robined over the 4 SWDGE queues (descriptor generation runs on the
Q7 core pair owning each queue, so queues overlap ~2-3.5x).
"""
import os
import sys

sys.path.insert(0, "/opt/trn_rl_repo")

import numpy as np
import ml_dtypes

import concourse.bass as bass
import concourse.bacc as bacc
import concourse.tile as tile
import concourse.mybir as mybir
from concourse import bass_utils
from concourse.library_config import mlp

N_CORES = 8
N_NODES = 100000
D_IN, D_H, D_OUT = 128, 64, 64
NSHARD = N_NODES // N_CORES          # 12500
TILE = 128
NT = (NSHARD + TILE - 1) // TILE     # 98
PADN = NT * TILE                     # 12544
PADN_ALL = N_CORES * PADN            # 100352
N_BUCKET = 2
B0_ROWS = 50176
BIAS = (32768, B0_ROWS + 32768)
GROUP = 4
GROUPS = [(t, min(t + GROUP, NT)) for t in range(0, NT, GROUP)]
N_GROUP = len(GROUPS)                # 25 (24x4 tiles + 1x2)

LAST_RESULT = None


def _host_prep(x, edge_index):
    src = np.asarray(edge_index[0], dtype=np.int64)
    dst = np.asarray(edge_index[1], dtype=np.int64)
    n = N_NODES

    deg = np.bincount(dst, minlength=n).astype(np.float64) + 1.0
    dinv = (1.0 / np.sqrt(deg)).astype(np.float32)

    # self-loops are applied densely on-device (one identity matmul per
    # tile) instead of occupying gather slots
    s_all = src
    d_all = dst

    core = d_all // NSHARD
    drem = d_all % NSHARD
    t_id = drem // TILE
    dloc = drem % TILE
    gsrc = (s_all // NSHARD) * PADN + (s_all % NSHARD)
    bkt = (gsrc >= B0_ROWS).astype(np.int64)

    key = (core * NT + t_id) * N_BUCKET + bkt
    order = np.argsort(key, kind="stable")
    key_s = key[order]
    gsrc_s = gsrc[order]
    dloc_s = dloc[order]

    ngroups = N_CORES * NT * N_BUCKET
    counts = np.bincount(key_s, minlength=ngroups).reshape(N_CORES, NT, N_BUCKET)
    nb = -(-counts.max(axis=0) // 128)                  # [NT, N_BUCKET] ceil
    nb = np.maximum(nb, 1)
    # each sub-gather ends at (tmid-1, b) or (t1-1, b); those (t,b) must end
    # with >=1 pad slot on every core (the gather ucode trims trailing
    # negative idxs, which would otherwise drop real edges).
    for (t0, t1) in GROUPS:
        tmid = (t0 + t1) // 2
        for tf in (tmid - 1, t1 - 1):
            for b in range(N_BUCKET):
                if (counts[:, tf, b] == nb[tf, b] * 128).any():
                    nb[tf, b] += 1

    # slot layout: per group of GROUP tiles: all b0 chunks (tile-major), then
    # all b1 chunks -> one contiguous gather dst region per (group, bucket).
    chunk_col = np.zeros((NT, N_BUCKET), np.int64)
    grp_nc = np.zeros(N_GROUP, np.int64)
    grp_base = np.zeros(N_GROUP, np.int64)
    grp_b_off = np.zeros((N_GROUP, N_BUCKET + 1), np.int64)
    pos = 0
    for g, (t0, t1) in enumerate(GROUPS):
        grp_base[g] = pos
        for b in range(N_BUCKET):
            grp_b_off[g, b] = pos - grp_base[g]
            for t in range(t0, t1):
                chunk_col[t, b] = pos
                pos += nb[t, b]
        grp_nc[g] = pos - grp_base[g]
        grp_b_off[g, N_BUCKET] = grp_nc[g]
    CHC = pos
    IDXC16 = CHC * 8

    grp_start = np.zeros(ngroups + 1, np.int64)
    np.cumsum(counts.reshape(-1), out=grp_start[1:])
    rank = np.arange(key_s.shape[0], dtype=np.int64) - grp_start[key_s]

    core_s = key_s // (NT * N_BUCKET)
    tb = key_s % (NT * N_BUCKET)
    t_s = tb // N_BUCKET
    b_s = tb % N_BUCKET

    slot = chunk_col[t_s, b_s] * 128 + rank
    ccol = slot // 128
    cpart = slot % 128

    idx_val = gsrc_s - np.where(b_s == 0, BIAS[0], BIAS[1])
    assert idx_val.min() >= -32768 and idx_val.max() <= 32767

    idx16_16 = np.zeros((N_CORES, 16, IDXC16), np.int16)
    idx16_16[core_s, slot % 16, slot // 16] = idx_val.astype(np.int16)
    idx16 = np.tile(idx16_16, (1, 8, 1))                # [cores, 128, IDXC16]

    dstloc = np.full((N_CORES, 128, CHC), 999.0, np.float32)
    dstloc[core_s, cpart, ccol] = dloc_s.astype(np.float32)

    dinv_cols = np.zeros((N_CORES, 128, NT), np.float32)
    node_grid = (
        np.arange(N_CORES)[:, None, None] * NSHARD
        + np.arange(NT)[None, None, :] * TILE
        + np.arange(128)[None, :, None]
    )
    local = np.arange(NT)[None, None, :] * TILE + np.arange(128)[None, :, None]
    valid = np.broadcast_to(local < NSHARD, node_grid.shape)
    dinv_cols[:] = np.where(valid, dinv[np.where(valid, node_grid, 0)], 0.0)

    xt = np.zeros((PADN_ALL, D_IN), np.float32)
    xs = (np.asarray(x, np.float32) * dinv[:, None]).reshape(N_CORES, NSHARD, D_IN)
    xt.reshape(N_CORES, PADN, D_IN)[:, :NSHARD] = xs
    xt = xt.astype(ml_dtypes.bfloat16)

    meta = dict(nb=nb, chunk_col=chunk_col, grp_nc=grp_nc, grp_base=grp_base,
                grp_b_off=grp_b_off, CHC=CHC, IDXC16=IDXC16)
    xts = np.ascontiguousarray(xt.reshape(N_CORES, PADN, D_IN))
    return xt, xts, idx16, dstloc, dinv_cols, meta


def _build_program(meta, b1_zero, b2_zero):
    nb = meta["nb"]
    chunk_col = meta["chunk_col"]
    grp_nc = meta["grp_nc"]
    grp_base = meta["grp_base"]
    grp_b_off = meta["grp_b_off"]
    CHC, IDXC16 = meta["CHC"], meta["IDXC16"]

    f32 = mybir.dt.float32
    bf16 = mybir.dt.bfloat16
    i16 = mybir.dt.int16
    nc = bacc.Bacc("TRN2", target_bir_lowering=False, debug=False,
                   num_devices=N_CORES, num_swdge_queues=4)

    xt_in = nc.dram_tensor("xt", [PADN_ALL, D_IN], bf16, kind="ExternalInput").ap()
    xs_in = nc.dram_tensor("xts", [PADN, D_IN], bf16, kind="ExternalInput").ap()
    idb_in = nc.dram_tensor("identb", [128, 128], bf16, kind="ExternalInput").ap()
    w1_in = nc.dram_tensor("W1", [D_IN, D_H], f32, kind="ExternalInput").ap()
    w2_in = nc.dram_tensor("W2", [D_H, D_OUT], f32, kind="ExternalInput").ap()
    b1_in = nc.dram_tensor("b1r", [128, D_H], f32, kind="ExternalInput").ap()
    b2_in = nc.dram_tensor("b2r", [128, D_OUT], f32, kind="ExternalInput").ap()
    io_in = nc.dram_tensor("iota", [128, 128], f32, kind="ExternalInput").ap()
    dv_in = nc.dram_tensor("dinv_cols", [128, NT], f32, kind="ExternalInput").ap()
    ix_in = nc.dram_tensor("idx16", [128, IDXC16], i16, kind="ExternalInput").ap()
    dl_in = nc.dram_tensor("dstloc", [128, CHC], f32, kind="ExternalInput").ap()
    out_t = nc.dram_tensor("out", [PADN, D_OUT], f32, kind="ExternalOutput").ap()

    rg = [list(range(N_CORES))]

    with tile.TileContext(nc) as tc:
        with tc.tile_pool(name="const", bufs=1) as constp, \
             tc.tile_pool(name="dram", bufs=1, space="DRAM") as dram, \
             tc.tile_pool(name="agg", bufs=2, space="PSUM") as aggp, \
             tc.tile_pool(name="tp", bufs=2, space="PSUM") as tpp, \
             tc.tile_pool(name="proj", bufs=2, space="PSUM") as projp, \
             tc.tile_pool(name="gat", bufs=5) as gatp, \
             tc.tile_pool(name="sel", bufs=3) as selp, \
             tc.tile_pool(name="sb", bufs=3) as sb:

            nc.gpsimd.load_library(mlp)

            w1 = constp.tile([D_IN, D_H], f32)
            nc.sync.dma_start(w1[:], w1_in[:])
            w2 = constp.tile([D_H, D_OUT], f32)
            nc.sync.dma_start(w2[:], w2_in[:])
            b1r = constp.tile([128, D_H], f32)
            nc.sync.dma_start(b1r[:], b1_in[:])
            b2r = constp.tile([128, D_OUT], f32)
            nc.sync.dma_start(b2r[:], b2_in[:])
            iota = constp.tile([128, 128], f32)
            nc.sync.dma_start(iota[:], io_in[:])
            identb = constp.tile([128, 128], bf16)
            nc.sync.dma_start(identb[:], idb_in[:])
            dvc = constp.tile([128, NT], f32)
            nc.sync.dma_start(dvc[:], dv_in[:])
            # idx + dstloc resident for both layers (same edges both layers)
            ixall = constp.tile([128, IDXC16], i16)
            nc.sync.dma_start(ixall[:], ix_in[:])
            dlall = constp.tile([128, CHC], f32)
            nc.sync.dma_start(dlall[:], dl_in[:])

            t2_shard = dram.tile([PADN, D_IN], bf16)
            t2_full = dram.tile([PADN_ALL, D_IN], bf16)

            def edge_pass(table_ap, width, layer):
                """table_ap: [PADN_ALL, 128] bf16 AP. width: 128 or 64."""
                tb0 = table_ap[BIAS[0]:B0_ROWS, :]
                tb1 = table_ap[BIAS[1]:PADN_ALL, :]
                def issue_gathers(g, t0g, t1g):
                    base = int(grp_base[g])
                    ncg = int(grp_nc[g])
                    G = gatp.tile([128, ncg, D_IN], bf16, tag="G")
                    # 4 sub-gathers per group (one per SWDGE queue): each
                    # bucket's chunk range split at a tile boundary so every
                    # sub-gather still ends in pad slots (trailing-trim safe).
                    qn = g
                    for b in range(N_BUCKET):
                        tmid = (t0g + t1g) // 2
                        lo = int(grp_b_off[g, b])
                        mid = int(chunk_col[tmid, b]) - base
                        hi = int(grp_b_off[g, b + 1])
                        for c0, c1 in ((lo, mid), (mid, hi)):
                            ncb = c1 - c0
                            if ncb == 0:
                                continue
                            nidx = ncb * 128
                            nc.gpsimd.dma_gather(
                                G[:, c0:c1, :],
                                tb0 if b == 0 else tb1,
                                ixall[:, (base + c0) * 8:(base + c1) * 8],
                                nidx, nidx, D_IN,
                                single_packet=False,
                                queue_num=qn % 4,
                            )
                            qn += 1
                    return G

                def consume_group(g, t0g, t1g, G):
                    base = int(grp_base[g])
                    for t in range(t0g, t1g):
                        nb0 = int(nb[t, 0])
                        nb1 = int(nb[t, 1])
                        nct = nb0 + nb1
                        l0 = int(chunk_col[t, 0]) - base
                        l1 = int(chunk_col[t, 1]) - base
                        dxt = sb.tile([128, width], bf16, tag="dx")
                        if layer == 1:
                            nc.sync.dma_start(
                                dxt[:], xs_in[t * 128:(t + 1) * 128, :])
                        else:
                            nc.sync.dma_start(
                                dxt[:],
                                t2_shard[t * 128:(t + 1) * 128, 0:D_H])
                        S0 = selp.tile([128, nb0, 128], bf16, tag="S0")
                        nc.vector.tensor_tensor(
                            out=S0[:],
                            in0=dlall[:, base + l0:base + l0 + nb0]
                                .to_broadcast([128, nb0, 128]),
                            in1=iota[:].unsqueeze(1).to_broadcast([128, nb0, 128]),
                            op=mybir.AluOpType.is_equal,
                        )
                        S1 = selp.tile([128, nb1, 128], bf16, tag="S1")
                        nc.vector.tensor_tensor(
                            out=S1[:],
                            in0=dlall[:, base + l1:base + l1 + nb1]
                                .to_broadcast([128, nb1, 128]),
                            in1=iota[:].unsqueeze(1).to_broadcast([128, nb1, 128]),
                            op=mybir.AluOpType.is_equal,
                        )
                        # aggT[f, d] = sum_e G[e, f] * S[e, d]  (G stationary,
                        # S moving) -> agg arrives pre-transposed for the
                        # feature-contraction projection matmul: no transpose.
                        aggT = aggp.tile([width, 128], f32, tag="agg")
                        # self-loop: aggT += dxt.T @ I (dense local rows)
                        nc.tensor.matmul(aggT[:], lhsT=dxt[:], rhs=identb[:],
                                         start=True, stop=False)
                        for i in range(nct):
                            S = S0[:, i, :] if i < nb0 else S1[:, i - nb0, :]
                            gc = (l0 + i) if i < nb0 else (l1 + i - nb0)
                            nc.tensor.matmul(
                                aggT[:], lhsT=G[:, gc, 0:width],
                                rhs=S,
                                start=False, stop=(i == nct - 1),
                            )
                        aggT_sb = sb.tile([width, 128], f32, tag="e1")
                        nc.scalar.copy(aggT_sb[:], aggT[:])
                        if layer == 1:
                            # t2 = dinv*relu(dinv*(agg @ W1) + b1)
                            proj = projp.tile([128, D_H], f32, tag="proj")
                            nc.tensor.matmul(proj[:], lhsT=aggT_sb[:], rhs=w1[:],
                                             start=True, stop=True)
                            if b1_zero:
                                hr = sb.tile([128, D_H], f32, tag="e5")
                                nc.scalar.activation(
                                    hr[:], proj[:],
                                    mybir.ActivationFunctionType.Relu,
                                    scale=dvc[:, t:t + 1])
                            else:
                                hv = sb.tile([128, D_H], f32, tag="e3")
                                nc.scalar.activation(
                                    hv[:], proj[:],
                                    mybir.ActivationFunctionType.Copy,
                                    scale=dvc[:, t:t + 1])
                                hb = sb.tile([128, D_H], f32, tag="e4")
                                nc.vector.tensor_add(hb[:], hv[:], b1r[:])
                                hr = sb.tile([128, D_H], f32, tag="e5")
                                nc.scalar.activation(
                                    hr[:], hb[:],
                                    mybir.ActivationFunctionType.Relu)
                            t2t = sb.tile([128, D_H], bf16, tag="e6")
                            nc.scalar.activation(
                                t2t[:], hr[:],
                                mybir.ActivationFunctionType.Copy,
                                scale=dvc[:, t:t + 1])
                            nc.sync.dma_start(
                                t2_shard[t * 128:(t + 1) * 128, 0:D_H], t2t[:])
                        else:
                            # out = dinv*(agg @ W2) + b2
                            o_ps = projp.tile([128, D_OUT], f32, tag="proj")
                            nc.tensor.matmul(o_ps[:], lhsT=aggT_sb[:], rhs=w2[:],
                                             start=True, stop=True)
                            ov = sb.tile([128, D_OUT], f32, tag="e3")
                            nc.scalar.activation(
                                ov[:], o_ps[:],
                                mybir.ActivationFunctionType.Copy,
                                scale=dvc[:, t:t + 1])
                            if b2_zero:
                                nc.sync.dma_start(
                                    out_t[t * 128:(t + 1) * 128, :], ov[:])
                            else:
                                ob = sb.tile([128, D_OUT], f32, tag="e4")
                                nc.vector.tensor_add(ob[:], ov[:], b2r[:])
                                nc.sync.dma_start(
                                    out_t[t * 128:(t + 1) * 128, :], ob[:])

                for gp in range(N_GROUP):
                    ga = issue_gathers(gp, *GROUPS[gp])
                    consume_group(gp, *GROUPS[gp], ga)

            edge_pass(xt_in, 128, layer=1)

            nc.gpsimd.collective_compute(
                "AllGather", mybir.AluOpType.bypass,
                ins=[t2_shard.opt()], outs=[t2_full.opt()],
                replica_groups=rg,
            )

            edge_pass(t2_full[:], D_H, layer=2)

    nc.compile()
    return nc


def kernel(x, edge_index, W1, b1, W2, b2):
    global LAST_RESULT
    x = np.asarray(x, np.float32)
    W1 = np.asarray(W1, np.float32)
    W2 = np.asarray(W2, np.float32)
    b1 = np.asarray(b1, np.float32)
    b2 = np.asarray(b2, np.float32)

    xt, xts, idx16, dstloc, dinv_cols, meta = _host_prep(x, edge_index)
    nc = _build_program(meta, bool(np.all(b1 == 0.0)),
                        bool(np.all(b2 == 0.0)))

    iota = np.tile(np.arange(128, dtype=np.float32), (128, 1))
    identb = np.eye(128, dtype=np.float32).astype(ml_dtypes.bfloat16)
    b1r = np.tile(b1[None, :], (128, 1)).astype(np.float32)
    b2r = np.tile(b2[None, :], (128, 1)).astype(np.float32)

    in_maps = []
    for k in range(N_CORES):
        in_maps.append({
            "xt": xt, "xts": xts[k], "identb": identb,
            "W1": W1, "W2": W2, "b1r": b1r, "b2r": b2r,
            "iota": iota,
            "dinv_cols": dinv_cols[k],
            "idx16": idx16[k],
            "dstloc": dstloc[k],
        })

    trace = bool(os.environ.get("BASS_TRACE"))
    res = bass_utils.run_bass_kernel_spmd(
        nc, in_maps, core_ids=list(range(N_CORES)), trace=trace)
    LAST_RESULT = res

    out = np.empty((N_NODES, D_OUT), np.float32)
    for k in range(N_CORES):
        out[k * NSHARD:(k + 1) * NSHARD] = res.results[k]["out"][:NSHARD]
    return out
